# revision 3
# baseline (speedup 1.0000x reference)
"""Complex self-attention (single-head) on 8 Trainium2 NeuronCores.

Problem: y = stack(re, im) of softmax(|q k^H|/sqrt(D)) @ v with complex
q/k/v projections of a complex input x.  B=8, N=1024, D=512, fp32 I/O.

Strategy
--------
Data-parallel over the batch: core c computes batch c entirely locally.

Per-core math (all matmuls fp16 operands, fp32 PSUM accumulation):
  * Host pre-transposes x (-> x^T [D, N]) and ships transposed / negated /
    pre-scaled weight variants, so no on-device transposes are needed.
  * sqrt(1/sqrt(D)) is folded into BOTH Wq and Wk (and bq, bk) so the
    score scale comes out exactly right with zero device work.
  * Projections accumulate complex parts directly in PSUM:
      q^T = Wq^T.T @ x^T   (lhsT = Wq^T chunk, rhs = x^T chunk)
      v   = x^T.T @ Wv^T   (lhsT = x^T chunk, rhs = Wv^T chunk)
  * Scores are computed TRANSPOSED, s^T[m, n] = sum_e k^T[e,m] q^T[e,n],
    so that E = exp(|s|) lands in [m, n] layout, which is exactly the
    lhsT layout the att@v matmul wants.  Softmax then needs NO max, NO
    transpose and NO vector reductions:
      - exp without max-subtraction is safe (|s| <= ~20); a constant
        EXP_SHIFT keeps exp() within fp16 range, and cancels in U/Z.
      - Z[n] = sum_m E[m,n] comes from a matmul against a ones column.
      - w = (E^T.T @ v) * (1/Z) with a per-partition scalar multiply.
"""

from contextlib import ExitStack

import numpy as np

import concourse.bass as bass
import concourse.mybir as mybir
import concourse.tile as tile
from concourse import bacc
from concourse.bass_utils import run_bass_kernel_spmd

B, N, D = 8, 1024, 512
P = 128
KC = D // P          # 4 contraction chunks of 128
MC = N // P          # 8 row chunks of 128
NH = 2               # halves of N (free dim <= 512 per matmul)
NCORES = 8
EXP_SHIFT = 11.0     # exp(|s| - SHIFT): keeps E in fp16 range; cancels in U/Z

f16 = mybir.dt.float16
f32 = mybir.dt.float32
AF = mybir.ActivationFunctionType


def emit(tc, ctx, nc, xr_d, xi_d, w_d, bqk_d, bv_d, out_d):
    singles = ctx.enter_context(tc.tile_pool(name="singles", bufs=1))
    ps = ctx.enter_context(tc.tile_pool(name="ps", bufs=6, space="PSUM"))
    psz = ctx.enter_context(tc.tile_pool(name="psz", bufs=2, space="PSUM"))
    tmp = ctx.enter_context(tc.tile_pool(name="tmp", bufs=4))
    outp = ctx.enter_context(tc.tile_pool(name="outp", bufs=4))

    # ---- inputs to SBUF -------------------------------------------------
    xr_sb = singles.tile([P, KC, N], f16)
    nc.sync.dma_start(out=xr_sb, in_=xr_d.rearrange("(c p) n -> p c n", p=P))
    xi_sb = singles.tile([P, KC, N], f16)
    nc.sync.dma_start(out=xi_sb, in_=xi_d.rearrange("(c p) n -> p c n", p=P))

    w_sb = singles.tile([P, 9, KC, D], f16)
    for t in range(9):
        nc.sync.dma_start(
            out=w_sb[:, t], in_=w_d[t].rearrange("(c p) e -> p c e", p=P)
        )

    bqk_sb = singles.tile([P, 5, KC], f32)
    nc.sync.dma_start(out=bqk_sb, in_=bqk_d)
    bv_sb = singles.tile([1, 2, D], f16)
    nc.sync.dma_start(out=bv_sb, in_=bv_d)

    ones_m = singles.tile([P, 1], f16)
    nc.vector.memset(ones_m, 1.0)
    ones_1 = singles.tile([1, P], f16)
    nc.vector.memset(ones_1, 1.0)
    shift_sb = singles.tile([P, 1], f32)
    nc.vector.memset(shift_sb, -EXP_SHIFT)

    # ---- persistent intermediates --------------------------------------
    # qk_sb slots: 0 qr^T, 1 qi^T, 2 -qi^T, 3 kr^T, 4 ki^T   (each [e, n])
    qk_sb = singles.tile([P, 5, KC, N], f16)
    v_sb = singles.tile([P, 2, MC, D], f16)      # v[, m-chunk, d] re/im
    et_sb = singles.tile([P, MC, N], f16)        # E^T[m, n] = exp(|s|-SHIFT)

    # ---- q/k projections ------------------------------------------------
    # (psum slot, W part a (x_re), W part b (x_im), copies [(dst, scale, bias)])
    qk_spec = [
        ((0, 2), [(0, 1.0, 0)]),            # qr = Wqr x_re - Wqi x_im
        ((1, 0), [(1, 1.0, 1), (2, -1.0, 2)]),  # qi (and -qi)
        ((3, 5), [(3, 1.0, 3)]),            # kr
        ((4, 3), [(4, 1.0, 4)]),            # ki
    ]
    for (wa, wb), copies in qk_spec:
        for ec in range(KC):
            for nh in range(NH):
                n0 = nh * 512
                pt = ps.tile([P, 512], f32, tag="b", name="pt")
                idx = 0
                for wi, xs in ((wa, xr_sb), (wb, xi_sb)):
                    for kc in range(KC):
                        nc.tensor.matmul(
                            pt,
                            lhsT=w_sb[:, wi, kc, ec * P:(ec + 1) * P],
                            rhs=xs[:, kc, n0:n0 + 512],
                            start=(idx == 0),
                            stop=(idx == 7),
                        )
                        idx += 1
                for dst, scl, bslot in copies:
                    nc.scalar.activation(
                        out=qk_sb[:, dst, ec, n0:n0 + 512],
                        in_=pt,
                        func=AF.Identity,
                        bias=bqk_sb[:, bslot, ec:ec + 1],
                        scale=scl,
                    )

    # ---- v projection ---------------------------------------------------
    for t, (wa, wb) in enumerate(((6, 8), (7, 6))):  # vr, vi
        for mc in range(MC):
            pt = ps.tile([P, 512], f32, tag="b", name="pt")
            idx = 0
            for xs, wi in ((xr_sb, wa), (xi_sb, wb)):
                for kc in range(KC):
                    nc.tensor.matmul(
                        pt,
                        lhsT=xs[:, kc, mc * P:(mc + 1) * P],
                        rhs=w_sb[:, wi, kc, :],
                        start=(idx == 0),
                        stop=False,
                    )
                    idx += 1
            # bias via rank-1 ones x bias matmul (stays in fp32 PSUM)
            nc.tensor.matmul(
                pt, lhsT=ones_1, rhs=bv_sb[:, t, :], start=False, stop=True
            )
            nc.vector.tensor_copy(out=v_sb[:, t, mc, :], in_=pt)

    # ---- scores + softmax numerator / AV, half by half ------------------
    def scores_half(nh):
        n0 = nh * 512
        for mc in range(MC):
            m0 = mc * P
            rt = ps.tile([P, 512], f32, tag="b", name="rt")
            it = ps.tile([P, 512], f32, tag="b", name="it")
            for out_t, pairs in ((rt, ((3, 0), (4, 2))), (it, ((3, 1), (4, 0)))):
                idx = 0
                for kt, qt in pairs:
                    for ec in range(KC):
                        nc.tensor.matmul(
                            out_t,
                            lhsT=qk_sb[:, kt, ec, m0:m0 + P],
                            rhs=qk_sb[:, qt, ec, n0:n0 + 512],
                            start=(idx == 0),
                            stop=(idx == 7),
                        )
                        idx += 1
            t1 = tmp.tile([P, 512], f32, tag="sq", name="t1")
            nc.scalar.activation(out=t1, in_=rt, func=AF.Square)
            t2 = tmp.tile([P, 512], f32, tag="sq", name="t2")
            nc.scalar.activation(out=t2, in_=it, func=AF.Square)
            u = tmp.tile([P, 512], f32, tag="u", name="u")
            nc.vector.tensor_add(u, t1, t2)
            a = tmp.tile([P, 512], f32, tag="a", name="a")
            nc.scalar.activation(out=a, in_=u, func=AF.Sqrt)
            nc.scalar.activation(
                out=et_sb[:, mc, n0:n0 + 512], in_=a, func=AF.Exp,
                bias=shift_sb,
            )

    def av_half(nh):
        for g in range(nh * 4, nh * 4 + 4):
            ur = ps.tile([P, 512], f32, tag="b", name="ur")
            ui = ps.tile([P, 512], f32, tag="b", name="ui")
            zp = psz.tile([P, 1], f32, tag="z", name="zp")
            for mc in range(MC):
                lh = et_sb[:, mc, g * P:(g + 1) * P]
                st, sp = mc == 0, mc == MC - 1
                nc.tensor.matmul(ur, lhsT=lh, rhs=v_sb[:, 0, mc, :], start=st, stop=sp)
                nc.tensor.matmul(ui, lhsT=lh, rhs=v_sb[:, 1, mc, :], start=st, stop=sp)
                nc.tensor.matmul(zp, lhsT=lh, rhs=ones_m, start=st, stop=sp)
            zr = tmp.tile([P, 1], f32, tag="zr", name="zr")
            nc.vector.reciprocal(zr, zp)
            for t, ut in ((0, ur), (1, ui)):
                ot = outp.tile([P, 512], f32, tag="o", name="ot")
                nc.vector.tensor_scalar_mul(ot, ut, zr)
                nc.sync.dma_start(out=out_d[t, g * P:(g + 1) * P, :], in_=ot)

    scores_half(0)
    av_half(0)
    scores_half(1)
    av_half(1)


def build_nc():
    nc = bacc.Bacc("TRN2", target_bir_lowering=False, debug=False)
    xr_d = nc.dram_tensor("xrT", [D, N], f16, kind="ExternalInput").ap()
    xi_d = nc.dram_tensor("xiT", [D, N], f16, kind="ExternalInput").ap()
    w_d = nc.dram_tensor("w9", [9, D, D], f16, kind="ExternalInput").ap()
    bqk_d = nc.dram_tensor("bqk", [P, 5, KC], f32, kind="ExternalInput").ap()
    bv_d = nc.dram_tensor("bv", [1, 2, D], f16, kind="ExternalInput").ap()
    out_d = nc.dram_tensor("out", [2, N, D], f32, kind="ExternalOutput").ap()
    with tile.TileContext(nc) as tc, ExitStack() as ctx:
        emit(tc, ctx, nc, xr_d, xi_d, w_d, bqk_d, bv_d, out_d)
    nc.compile()
    return nc


def make_in_maps(inputs):
    sc = float((1.0 / np.sqrt(D)) ** 0.5)

    def t16(a, s=1.0):
        return np.ascontiguousarray(a.T * s).astype(np.float16)

    w9 = np.stack([
        t16(inputs["Wq_re"], sc), t16(inputs["Wq_im"], sc), t16(inputs["Wq_im"], -sc),
        t16(inputs["Wk_re"], sc), t16(inputs["Wk_im"], sc), t16(inputs["Wk_im"], -sc),
        t16(inputs["Wv_re"]), t16(inputs["Wv_im"]), t16(inputs["Wv_im"], -1.0),
    ])
    bqk = np.stack([
        inputs["bq_re"] * sc, inputs["bq_im"] * sc, -inputs["bq_im"] * sc,
        inputs["bk_re"] * sc, inputs["bk_im"] * sc,
    ]).astype(np.float32)                       # [5, 512]
    bqk = bqk.reshape(5, KC, P).transpose(2, 0, 1).copy()  # [128, 5, KC]
    bv = np.stack([inputs["bv_re"], inputs["bv_im"]])[None].astype(np.float16)

    xrT = inputs["x_re"].transpose(0, 2, 1).astype(np.float16)  # [B, D, N]
    xiT = inputs["x_im"].transpose(0, 2, 1).astype(np.float16)
    return [
        {
            "xrT": np.ascontiguousarray(xrT[c]),
            "xiT": np.ascontiguousarray(xiT[c]),
            "w9": w9,
            "bqk": bqk,
            "bv": bv,
        }
        for c in range(NCORES)
    ]


_NC_CACHE = None


def get_nc():
    global _NC_CACHE
    if _NC_CACHE is None:
        _NC_CACHE = build_nc()
    return _NC_CACHE


def kernel(**inputs) -> np.ndarray:
    nc = get_nc()
    in_maps = make_in_maps(inputs)
    res = run_bass_kernel_spmd(nc, in_maps, core_ids=list(range(NCORES)))
    return np.stack([res.results[c]["out"] for c in range(NCORES)], axis=1)


# revision 23
# speedup vs baseline: 1.1425x; 1.1425x over previous
"""Complex self-attention (single-head) on 8 Trainium2 NeuronCores.

Problem: y = stack(re, im) of softmax(|q k^H|/sqrt(D)) @ v with complex
q/k/v projections of a complex input x.  B=8, N=1024, D=512, fp32 I/O.

Strategy
--------
Data-parallel over the batch: core c computes batch c entirely locally.

Per-core math (all matmuls fp16 operands, fp32 PSUM accumulation):
  * Host pre-transposes x (-> x^T [D, N]) and ships transposed / negated /
    pre-scaled weight variants, so no on-device transposes are needed.
  * sqrt(1/sqrt(D)) is folded into BOTH Wq and Wk (and bq, bk) so the
    score scale comes out exactly right with zero device work.
  * Projections accumulate complex parts directly in PSUM:
      q^T = Wq^T.T @ x^T   (lhsT = Wq^T chunk, rhs = x^T chunk)
      v   = x^T.T @ Wv^T   (lhsT = x^T chunk, rhs = Wv^T chunk)
  * Scores are computed TRANSPOSED, s^T[m, n] = sum_e k^T[e,m] q^T[e,n],
    so that E = exp(|s|) lands in [m, n] layout, which is exactly the
    lhsT layout the att@v matmul wants.  Softmax then needs NO max, NO
    transpose and NO vector reductions:
      - exp without max-subtraction is safe (|s| <= ~20); a constant
        EXP_SHIFT keeps exp() within fp16 range, and cancels in U/Z.
      - Z[n] = sum_m E[m,n] comes from a matmul against a ones column.
      - w = (E^T.T @ v) * (1/Z) with a per-partition scalar multiply.
"""

from contextlib import ExitStack

import numpy as np

import concourse.bass as bass
import concourse.mybir as mybir
import concourse.tile as tile
from concourse import bacc
from concourse.bass_utils import run_bass_kernel_spmd

B, N, D = 8, 1024, 512
P = 128
KC = D // P          # 4 contraction chunks of 128
MC = N // P          # 8 row chunks of 128
NH = 2               # halves of N (free dim <= 512 per matmul)
NCORES = 8
EXP_SHIFT = 11.0     # exp(|s| - SHIFT): keeps E in fp16 range; cancels in U/Z

f16 = mybir.dt.float16
f32 = mybir.dt.float32
AF = mybir.ActivationFunctionType


def emit(tc, ctx, nc, xr_d, xi_d, w_d, bqk_d, bv_d, out_d):
    singles = ctx.enter_context(tc.tile_pool(name="singles", bufs=1))
    ps = ctx.enter_context(tc.tile_pool(name="ps", bufs=6, space="PSUM"))
    psz = ctx.enter_context(tc.tile_pool(name="psz", bufs=2, space="PSUM"))
    tmp = ctx.enter_context(tc.tile_pool(name="tmp", bufs=4))
    outp = ctx.enter_context(tc.tile_pool(name="outp", bufs=4))

    # ---- inputs to SBUF -------------------------------------------------
    # x on the SP HWDGE ring, weights on the ACT ring (the two HW rings run
    # in parallel); both split per contraction-chunk so matmuls start early.
    xr_sb = singles.tile([P, KC, N], f16)
    xi_sb = singles.tile([P, KC, N], f16)
    xs_sb = singles.tile([P, KC, N], f16)    # x_re + x_im (computed on DVE)
    xr_r = xr_d.rearrange("(c p) n -> p c n", p=P)
    xi_r = xi_d.rearrange("(c p) n -> p c n", p=P)
    w_sb = singles.tile([P, 9, KC, D], f16)
    w_r = [w_d[t].rearrange("(c p) e -> p c e", p=P) for t in range(9)]

    bqk_sb = singles.tile([P, 4, KC], f32)
    nc.sync.dma_start(out=bqk_sb, in_=bqk_d)
    # single HWDGE ring (SP): interleave weight/x chunks in exact first-use
    # order so the projection matmuls are never waiting on a later transfer.
    for kc in range(KC):
        nc.sync.dma_start(out=w_sb[:, 0, kc], in_=w_r[0][:, kc])
        nc.sync.dma_start(out=xr_sb[:, kc], in_=xr_r[:, kc])
    for kc in range(KC):
        nc.sync.dma_start(out=w_sb[:, 1, kc], in_=w_r[1][:, kc])
        nc.sync.dma_start(out=xi_sb[:, kc], in_=xi_r[:, kc])
        nc.vector.tensor_add(xs_sb[:, kc], xr_sb[:, kc], xi_sb[:, kc])
    for t in (2, 3, 4, 5, 6, 8, 7):
        for kc in range(KC):
            nc.sync.dma_start(out=w_sb[:, t, kc], in_=w_r[t][:, kc])
    # v bias broadcast across partitions (fp32): fused into the v PSUM->SBUF copy
    bv_bc = singles.tile([P, 2, D], f32)
    nc.gpsimd.dma_start(
        out=bv_bc,
        in_=bass.AP(tensor=bv_d.tensor, offset=bv_d.offset, ap=[[0, P]] + list(bv_d.ap[1:])),
    )

    ones_m = singles.tile([P, 1], f16)
    nc.vector.memset(ones_m, 1.0)
    shift_sb = singles.tile([P, 1], f32)
    nc.vector.memset(shift_sb, -EXP_SHIFT)

    # ---- persistent intermediates --------------------------------------
    # qk_sb slots: 0 qr^T, 1 qi^T, 2 -qi^T, 3 kr^T, 4 ki^T   (each [e, n])
    qk_sb = singles.tile([P, 5, KC, N], f16)
    v_sb = singles.tile([P, 2, MC, D], f16)      # v[, m-chunk, d] re/im
    et_sb = singles.tile([P, MC, N], f16)        # E^T[m, n] = exp(|s|-SHIFT)

    # ---- q/k projections (Karatsuba: 3 products per complex matmul) -----
    # T1 = Wr x_re, T2 = Wi x_im, T3 = Wsum x_sum;
    # re = T1 - T2 + b_r;  im = T3 - T1 - T2 + b_i.
    # (w slots r/i/sum, bias slots b_r / b_r+b_i, dst slots re/im/neg-im)
    qk_spec = [
        ((0, 1, 2), (0, 1), (0, 1, 2)),      # q (also writes -qi)
        ((3, 4, 5), (2, 3), (3, 4, None)),   # k
    ]
    t1bp = ctx.enter_context(tc.tile_pool(name="t1bp", bufs=9))
    t12p = ctx.enter_context(tc.tile_pool(name="t12p", bufs=9))
    tiles = [(ec, nh) for ec in range(KC) for nh in range(NH)]
    for (w_r, w_i, w_s), (b_r, b_s), (d_r, d_i, d_n) in qk_spec:
        # phase A: T1 = Wr x_re -> t1b = T1 + b_r   (only needs Wr + x_re)
        t1bs, t12s = {}, {}
        for ec, nh in tiles:
            n0, e0 = nh * 512, ec * P
            pt = ps.tile([P, 512], f32, tag="b", name="pt")
            for kc in range(KC):
                nc.tensor.matmul(
                    pt, lhsT=w_sb[:, w_r, kc, e0:e0 + P],
                    rhs=xr_sb[:, kc, n0:n0 + 512],
                    start=(kc == 0), stop=(kc == KC - 1),
                )
            t1b = t1bp.tile([P, 512], f32, tag="t1b", name="t1b")
            nc.scalar.activation(
                out=t1b, in_=pt, func=AF.Identity,
                bias=bqk_sb[:, b_r, ec:ec + 1],
            )
            t1bs[ec, nh] = t1b
        # phase B: T2 = Wi x_im -> re = t1b - T2 ; t12b = T2 + t1b
        for ec, nh in tiles:
            n0, e0 = nh * 512, ec * P
            pt = ps.tile([P, 512], f32, tag="b", name="pt")
            for kc in range(KC):
                nc.tensor.matmul(
                    pt, lhsT=w_sb[:, w_i, kc, e0:e0 + P],
                    rhs=xi_sb[:, kc, n0:n0 + 512],
                    start=(kc == 0), stop=(kc == KC - 1),
                )
            nc.vector.scalar_tensor_tensor(
                out=qk_sb[:, d_r, ec, n0:n0 + 512],
                in0=pt, scalar=-1.0, in1=t1bs[ec, nh],
                op0=mybir.AluOpType.mult, op1=mybir.AluOpType.add,
            )
            t12b = t12p.tile([P, 512], f32, tag="t12b", name="t12b")
            nc.vector.tensor_add(t12b, pt, t1bs[ec, nh])
            t12s[ec, nh] = t12b
        # phase C: T3 = Wsum x_sum -> im = (T3 + b_sum) - t12b  (and -im)
        for ec, nh in tiles:
            n0, e0 = nh * 512, ec * P
            pt = ps.tile([P, 512], f32, tag="b", name="pt")
            for kc in range(KC):
                nc.tensor.matmul(
                    pt, lhsT=w_sb[:, w_s, kc, e0:e0 + P],
                    rhs=xs_sb[:, kc, n0:n0 + 512],
                    start=(kc == 0), stop=(kc == KC - 1),
                )
            nc.vector.scalar_tensor_tensor(
                out=qk_sb[:, d_i, ec, n0:n0 + 512],
                in0=pt, scalar=bqk_sb[:, b_s, ec:ec + 1], in1=t12s[ec, nh],
                op0=mybir.AluOpType.add, op1=mybir.AluOpType.subtract,
            )
            if d_n is not None:  # -qi from qi (fast fp16 sbuf pass)
                nc.vector.tensor_scalar(
                    qk_sb[:, d_n, ec, n0:n0 + 512],
                    qk_sb[:, d_i, ec, n0:n0 + 512],
                    -1.0, None, mybir.AluOpType.mult,
                )

    # ---- v projection ---------------------------------------------------
    for t, (wa, wb) in enumerate(((6, 8), (7, 6))):  # vr, vi
        for mc in range(MC):
            pt = ps.tile([P, 512], f32, tag="b", name="pt")
            idx = 0
            for xs, wi in ((xr_sb, wa), (xi_sb, wb)):
                for kc in range(KC):
                    nc.tensor.matmul(
                        pt,
                        lhsT=xs[:, kc, mc * P:(mc + 1) * P],
                        rhs=w_sb[:, wi, kc, :],
                        start=(idx == 0),
                        stop=(idx == 7),
                    )
                    idx += 1
            nc.vector.tensor_add(v_sb[:, t, mc, :], pt, bv_bc[:, t, :])

    # ---- scores + softmax numerator / AV, half by half ------------------
    def scores_half(nh):
        n0 = nh * 512
        for mc in range(MC):
            m0 = mc * P
            rt = ps.tile([P, 512], f32, tag="b", name="rt")
            it = ps.tile([P, 512], f32, tag="b", name="it")
            for out_t, pairs in ((rt, ((3, 0), (4, 2))), (it, ((3, 1), (4, 0)))):
                idx = 0
                for kt, qt in pairs:
                    for ec in range(KC):
                        nc.tensor.matmul(
                            out_t,
                            lhsT=qk_sb[:, kt, ec, m0:m0 + P],
                            rhs=qk_sb[:, qt, ec, n0:n0 + 512],
                            start=(idx == 0),
                            stop=(idx == 7),
                        )
                        idx += 1
            t1 = tmp.tile([P, 512], f32, tag="sq", name="t1")
            nc.scalar.activation(out=t1, in_=rt, func=AF.Square)
            t2 = tmp.tile([P, 512], f32, tag="sq", name="t2")
            nc.scalar.activation(out=t2, in_=it, func=AF.Square)
            u = tmp.tile([P, 512], f32, tag="u", name="u")
            nc.vector.tensor_add(u, t1, t2)
            a = tmp.tile([P, 512], f32, tag="a", name="a")
            nc.scalar.activation(out=a, in_=u, func=AF.Sqrt)
            nc.scalar.activation(
                out=et_sb[:, mc, n0:n0 + 512], in_=a, func=AF.Exp,
                bias=shift_sb,
            )

    def av_half(nh):
        for g in range(nh * 4, nh * 4 + 4):
            ur = ps.tile([P, 512], f32, tag="b", name="ur")
            ui = ps.tile([P, 512], f32, tag="b", name="ui")
            zp = psz.tile([P, 1], f32, tag="z", name="zp")
            for mc in range(MC):
                lh = et_sb[:, mc, g * P:(g + 1) * P]
                st, sp = mc == 0, mc == MC - 1
                nc.tensor.matmul(ur, lhsT=lh, rhs=v_sb[:, 0, mc, :], start=st, stop=sp)
                nc.tensor.matmul(ui, lhsT=lh, rhs=v_sb[:, 1, mc, :], start=st, stop=sp)
                nc.tensor.matmul(zp, lhsT=lh, rhs=ones_m, start=st, stop=sp)
            zr = tmp.tile([P, 1], f32, tag="zr", name="zr")
            nc.vector.reciprocal(zr, zp)
            # re on DVE + SP DMA ring, im on ACT + its DMA ring: the two
            # normalize+store chains of a chunk run in parallel at the tail
            o0 = outp.tile([P, 512], f32, tag="o", name="o0")
            nc.vector.tensor_scalar_mul(o0, ur, zr)
            nc.sync.dma_start(out=out_d[0, g * P:(g + 1) * P, :], in_=o0)
            o1 = outp.tile([P, 512], f32, tag="o", name="o1")
            nc.scalar.activation(out=o1, in_=ui, func=AF.Copy, scale=zr)
            nc.scalar.dma_start(out=out_d[1, g * P:(g + 1) * P, :], in_=o1)

    scores_half(0)
    av_half(0)
    scores_half(1)
    av_half(1)


def build_nc():
    nc = bacc.Bacc("TRN2", target_bir_lowering=False, debug=False)
    xr_d = nc.dram_tensor("xrT", [D, N], f16, kind="ExternalInput").ap()
    xi_d = nc.dram_tensor("xiT", [D, N], f16, kind="ExternalInput").ap()
    w_d = nc.dram_tensor("w9", [9, D, D], f16, kind="ExternalInput").ap()
    bqk_d = nc.dram_tensor("bqk", [P, 4, KC], f32, kind="ExternalInput").ap()
    bv_d = nc.dram_tensor("bv", [1, 2, D], f32, kind="ExternalInput").ap()
    out_d = nc.dram_tensor("out", [2, N, D], f32, kind="ExternalOutput").ap()
    with tile.TileContext(nc) as tc, ExitStack() as ctx:
        emit(tc, ctx, nc, xr_d, xi_d, w_d, bqk_d, bv_d, out_d)
    nc.compile()
    return nc


def make_in_maps(inputs):
    sc = float((1.0 / np.sqrt(D)) ** 0.5)

    def t16(a, s=1.0):
        return np.ascontiguousarray(a.T * s).astype(np.float16)

    # w slots: q r/i/sum (scaled), k r/i/sum (scaled), v r / i / -i
    w9 = np.stack([
        t16(inputs["Wq_re"], sc), t16(inputs["Wq_im"], sc),
        t16(inputs["Wq_re"] + inputs["Wq_im"], sc),
        t16(inputs["Wk_re"], sc), t16(inputs["Wk_im"], sc),
        t16(inputs["Wk_re"] + inputs["Wk_im"], sc),
        t16(inputs["Wv_re"]), t16(inputs["Wv_im"]), t16(inputs["Wv_im"], -1.0),
    ])
    bqk = np.stack([
        inputs["bq_re"] * sc, (inputs["bq_re"] + inputs["bq_im"]) * sc,
        inputs["bk_re"] * sc, (inputs["bk_re"] + inputs["bk_im"]) * sc,
    ]).astype(np.float32)                       # [4, 512]
    bqk = bqk.reshape(4, KC, P).transpose(2, 0, 1).copy()  # [128, 4, KC]
    bv = np.stack([inputs["bv_re"], inputs["bv_im"]])[None].astype(np.float32)

    xrT = inputs["x_re"].transpose(0, 2, 1).astype(np.float16)  # [B, D, N]
    xiT = inputs["x_im"].transpose(0, 2, 1).astype(np.float16)
    return [
        {
            "xrT": np.ascontiguousarray(xrT[c]),
            "xiT": np.ascontiguousarray(xiT[c]),
            "w9": w9,
            "bqk": bqk,
            "bv": bv,
        }
        for c in range(NCORES)
    ]


_NC_CACHE = None


def get_nc():
    global _NC_CACHE
    if _NC_CACHE is None:
        _NC_CACHE = build_nc()
    return _NC_CACHE


def kernel(**inputs) -> np.ndarray:
    nc = get_nc()
    in_maps = make_in_maps(inputs)
    res = run_bass_kernel_spmd(nc, in_maps, core_ids=list(range(NCORES)))
    return np.stack([res.results[c]["out"] for c in range(NCORES)], axis=1)
